# revision 14
# baseline (speedup 1.0000x reference)
"""ConvMod3d (StyleGAN-style modulated 3x3x3 conv, N=4 groups) on 8 trn2 cores.

Sharding: 8 shards = 4 samples x 2 depth-halves. Each core convolves a
25-plane input slab (64ch x 48x48) against its sample's modulated 64x64x27
weights, producing 23 output planes. Style modulation/demodulation of the
tiny weight tensor happens on host; the conv (99.8% of FLOPs) on device.

Per output plane d': 27 taps, each a [Cin=64 -> Cout=64] matmul over the
flattened 48x48 plane with a shifted read offset; invalid edge columns
(w'>=46, h'>=46) are computed and discarded on the host side.

PE packing (trn2 constraints: row tiling crashes the device; alternating
contract sizes back-to-back costs 2.2x, so same-contract matmuls are kept
contiguous). Taps are packed two-per-matmul on the contraction dim via
stacked SBUF windows:
- W[p]  = plane p (partitions 0-63) | plane p+1 (64-127): fuses the
  (kd=0,kd=1) tap pairs -> 9 contract-128 streams per output plane.
- W2[p] = plane p | plane p shifted +48 cols (one h row): fuses the
  (kd=2, kh=0/1) pairs -> 3 contract-128 streams; the 3 (kd=2,kh=2)
  taps stay contract-64 on W2's lower half. (A third +1-col-shift
  family packing those too was tried and lost: its extra HBM traffic
  stalls the PE on window waits.)
Two output planes run concurrently on PE col strips (plane A accumulates
in one PSUM bank partitions 0-63, plane B in another, partitions 64-127).
Matmuls in bf16 (fp32 PSUM accumulation).
"""

import time

import numpy as np
import ml_dtypes

import concourse.bacc as bacc
import concourse.bass as bass
import concourse.tile as tile
from concourse import mybir
from concourse.bass_utils import run_bass_kernel_spmd

EPS = 1e-8
N, CIN, COUT = 4, 64, 64
DHW, K = 48, 3
DOUT = DHW - K + 1          # 46
HALF = DOUT // 2            # 23 output planes per core
P_IN = HALF + K - 1         # 25 input planes per core
PLANE = DHW * DHW           # 2304
PAD_COLS = 192              # tail slack so shifted reads stay in-bounds
XS_COLS = P_IN * PLANE + PAD_COLS
WCOLS = PLANE + PAD_COLS - 64   # window columns (2432); max offset used 98+2207
PLANE_OUT = (DHW - 2) * DHW     # 2208 computed output cols (h' rows 0-45)
NTAPS = K * K * K           # 27
GROUP = 2                   # output planes per group (PSUM col strips)
NGROUPS = (HALF + GROUP - 1) // GROUP
CHUNKS = [(0, 512), (512, 512), (1024, 512), (1536, 512), (2048, 160)]
NCORES = 8
NWBLK = 15                  # weight blocks of 64 cols

F32 = mybir.dt.float32
MM_DT = mybir.dt.bfloat16
NP_MM = np.dtype(ml_dtypes.bfloat16)

_CACHE = {}
LAST_RESULTS = None  # BassKernelResults of the most recent device run


def _build_bass():
    nc = bacc.Bacc()
    xs = nc.declare_dram_parameter("xs", [CIN, XS_COLS], MM_DT, isOutput=False)
    # weight blocks duplicated to 128 cols ([w_j | w_j]) so one LDWEIGHTS
    # fills both PE col strips
    wt = nc.declare_dram_parameter("wt", [128, NWBLK * 128], MM_DT, isOutput=False)
    bt = nc.declare_dram_parameter("bt", [128, 1], F32, isOutput=False)
    y = nc.declare_dram_parameter(
        "y", [NGROUPS, GROUP * 64, PLANE_OUT], F32, isOutput=True)

    with tile.TileContext(nc) as tc:
        with (
            tc.tile_pool(name="const", bufs=1) as cpool,
            tc.tile_pool(name="xpool", bufs=16) as xpool,
            tc.tile_pool(name="opool", bufs=3) as opool,
            tc.tile_pool(name="ppool", bufs=8, space="PSUM") as ppool,
        ):
            wtile = cpool.tile([128, NWBLK * 128], MM_DT)
            nc.sync.dma_start(out=wtile[:, :], in_=wt[:, :])
            btile = cpool.tile([128, 1], F32)
            nc.sync.dma_start(out=btile[:, :], in_=bt[:, :])

            windows = {}

            UPSHIFT = {"w": PLANE, "w2": DHW}

            def load_window(fam, p, split=False):
                # upper half holds the lower plane shifted by UPSHIFT[fam].
                # split=True loads in column halves so early matmuls (which
                # only touch low columns) start before the full window lands.
                key = (fam, p)
                if key in windows or p >= P_IN:
                    return
                xw = xpool.tile([128, WCOLS], MM_DT, tag="xw", name="xw")
                base = p * PLANE
                up = base + UPSHIFT[fam]
                cuts = [0, 1280, WCOLS] if split else [0, WCOLS]
                for a, b in zip(cuts, cuts[1:]):
                    nc.sync.dma_start(out=xw[0:64, a:b],
                                      in_=xs[:, base + a:base + b])
                    if up + WCOLS <= XS_COLS:
                        nc.sync.dma_start(out=xw[64:128, a:b],
                                          in_=xs[:, up + a:up + b])
                windows[key] = xw

            def ensure_group_windows(g, split=False):
                if g >= NGROUPS:
                    return
                for d in range(g * GROUP, min(HALF, (g + 1) * GROUP)):
                    load_window("w", d, split=split)
                    load_window("w2", d + 2, split=split)

            # Group-0 windows: issue ONLY the low-column cut before the
            # first matmuls. A matmul's DMA wait is a coarse semaphore
            # threshold covering every DMA issued before it in program
            # order, so anything issued earlier delays the first matmul
            # (measured: 2.9MB up-front -> first MM at ~13us). The rest of
            # g0's columns and the g1/g2/g3 prefetches are issued between
            # the first group's chunk iterations instead.
            XCUT = 1280
            g0wins = [("w", 0), ("w", 1), ("w2", 2), ("w2", 3)]
            for fam, p in g0wins:
                windows[(fam, p)] = xpool.tile([128, WCOLS], MM_DT,
                                               tag="xw", name="xw")

            def g0_cut(a, b):
                for fam, p in g0wins:
                    xw = windows[(fam, p)]
                    base = p * PLANE
                    up = base + UPSHIFT[fam]
                    nc.sync.dma_start(out=xw[0:64, a:b],
                                      in_=xs[:, base + a:base + b])
                    if up + WCOLS <= XS_COLS:
                        nc.sync.dma_start(out=xw[64:128, a:b],
                                          in_=xs[:, up + a:up + b])

            g0_cut(0, XCUT)

            for grp in range(NGROUPS):
                dps = [d for d in range(grp * GROUP, (grp + 1) * GROUP)
                       if d < HALF]
                if grp > 0:
                    ensure_group_windows(grp + 3)
                nparts = 64 * len(dps)

                ot = opool.tile([128, PLANE_OUT], F32, tag="ot")
                for cidx, (c0, csz) in enumerate(CHUNKS):
                    pss = [ppool.tile([128, 512], F32, tag="ps", name="ps")
                           for _ in dps]
                    # j 0-8 fused kd01 (c128, W[dp], off kh*48+kw);
                    # j 9-11 fused kd2 kh01 (c128, W2[dp+2], off kw);
                    # j 12-14 kd2 kh2 (c64, W2[dp+2] lower, off 96+kw).
                    # Same-contract matmuls contiguous; serpentine the
                    # kind order across chunks so chunk boundaries don't
                    # add a contract-size switch.
                    # One explicit 128-col LDWEIGHTS per j loads BOTH col
                    # strips' (identical) weights; the two matmuls are
                    # marked ldweights=False so they skip their own weight
                    # reload. Halves the LDWEIGHTS issue load on the
                    # Tensor queue, which otherwise throttles the PE
                    # (~216ns of LDW issue vs ~213ns of rhs streaming per
                    # matmul pair).
                    jorder = list(range(NWBLK))
                    if cidx % 2 == 1:
                        jorder = jorder[12:] + jorder[:12]
                    for jj, j in enumerate(jorder):
                        rows = 128 if j < 12 else 64
                        nc.tensor.ldweights(
                            wtile[0:rows, j * 128:(j + 1) * 128],
                            tile_position=(0, 0),
                        )
                        for ci in range(len(dps)):
                            dst = pss[ci][ci * 64:(ci + 1) * 64, 0:csz]
                            if j < 9:
                                kh, kw = divmod(j, 3)
                                win = windows[("w", dps[ci])]
                                off = kh * DHW + kw + c0
                            elif j < 12:
                                kw = j - 9
                                win = windows[("w2", dps[ci] + 2)]
                                off = kw + c0
                            else:
                                kw = j - 12
                                win = windows[("w2", dps[ci] + 2)]
                                off = 2 * DHW + kw + c0
                            mm = nc.tensor.matmul(
                                dst,
                                wtile[0:rows,
                                      j * 128 + ci * 64:j * 128 + ci * 64 + 64],
                                win[0:rows, off:off + csz],
                                start=(jj == 0),
                                stop=(jj == NWBLK - 1),
                            )
                            mm.ins.ldweights = False
                    if grp == 0:
                        # staged loads: anything issued before the first
                        # matmuls inflates their DMA-wait threshold, so
                        # the rest of the input streams in behind chunk 0
                        if cidx == 0:
                            g0_cut(XCUT, WCOLS)
                        elif cidx == 1:
                            ensure_group_windows(1)
                        elif cidx == 2:
                            ensure_group_windows(2)
                        elif cidx == 3:
                            ensure_group_windows(3)
                    for ci in range(len(dps)):
                        nc.scalar.activation(
                            ot[ci * 64:(ci + 1) * 64, c0:c0 + csz],
                            pss[ci][ci * 64:(ci + 1) * 64, 0:csz],
                            mybir.ActivationFunctionType.Identity,
                            bias=btile[ci * 64:(ci + 1) * 64, :],
                        )
                    if grp == NGROUPS - 1:
                        # last group: per-chunk store on the idle Vector
                        # queue so the final transfer overlaps the
                        # remaining chunks' matmuls
                        nc.gpsimd.dma_start(
                            out=y[grp, 0:nparts, c0:c0 + csz],
                            in_=ot[0:nparts, c0:c0 + csz])
                if grp < NGROUPS - 1:
                    nc.gpsimd.dma_start(out=y[grp, 0:nparts, :],
                                        in_=ot[0:nparts, :])
    nc.compile()
    return nc


def _prep_in_maps(x, s, style_weight, style_bias, weight, bias):
    style = s @ style_weight.T + style_bias                      # [N, Cin]
    wm = weight[None] * style[:, None, :, None, None, None]      # [N,Co,Ci,k,k,k]
    wm = wm * (1.0 / np.sqrt((wm * wm).sum(axis=(2, 3, 4, 5), keepdims=True) + EPS))
    wk = wm.transpose(0, 2, 3, 4, 5, 1)                          # [N,Ci,kd,kh,kw,Co]
    wfull = np.zeros((N, 128, NWBLK * COUT), np.float32)
    for j in range(9):
        kh, kw = divmod(j, 3)
        wfull[:, 0:64, j * 64:(j + 1) * 64] = wk[:, :, 0, kh, kw, :]
        wfull[:, 64:128, j * 64:(j + 1) * 64] = wk[:, :, 1, kh, kw, :]
    for kw in range(3):
        j = 9 + kw
        wfull[:, 0:64, j * 64:(j + 1) * 64] = wk[:, :, 2, 0, kw, :]
        wfull[:, 64:128, j * 64:(j + 1) * 64] = wk[:, :, 2, 1, kw, :]
    for kw in range(3):
        j = 12 + kw
        wfull[:, 0:64, j * 64:(j + 1) * 64] = wk[:, :, 2, 2, kw, :]
    # duplicate each 64-col block to [w_j | w_j] (one 128-col LDWEIGHTS
    # serves both PE col strips)
    wfull = wfull.reshape(N, 128, NWBLK, 1, COUT)
    wfull = np.broadcast_to(wfull, (N, 128, NWBLK, 2, COUT))
    wfull = wfull.reshape(N, 128, NWBLK * 128)
    wfull = np.ascontiguousarray(wfull.astype(NP_MM))
    bt = np.ascontiguousarray(
        np.tile(bias[:, None], (2, 1)), dtype=np.float32)        # [128,1]

    in_maps = []
    for core in range(NCORES):
        n, h = divmod(core, 2)
        d0 = h * HALF
        xsl = x[n, :, d0:d0 + P_IN].reshape(CIN, P_IN * PLANE)
        xsl = np.concatenate(
            [xsl, np.zeros((CIN, PAD_COLS), np.float32)], axis=1)
        in_maps.append({
            "xs": np.ascontiguousarray(xsl.astype(NP_MM)),
            "wt": wfull[n],
            "bt": bt,
        })
    return in_maps


def _gather(results):
    y = np.empty((N, COUT, DOUT, DOUT, DOUT), np.float32)
    for core in range(NCORES):
        n, h = divmod(core, 2)
        planes = results[core]["y"].reshape(
            NGROUPS * GROUP, COUT, DHW - 2, DHW)[:HALF]
        y[n, :, h * HALF:(h + 1) * HALF] = (
            planes[:, :, :, :DOUT].transpose(1, 0, 2, 3))
    return y


def kernel(x, s, style_weight, style_bias, weight, bias):
    global LAST_RESULTS
    x = np.asarray(x, np.float32)
    s = np.asarray(s, np.float32)
    style_weight = np.asarray(style_weight, np.float32)
    style_bias = np.asarray(style_bias, np.float32)
    weight = np.asarray(weight, np.float32)
    bias = np.asarray(bias, np.float32)

    if "nc" not in _CACHE:
        _CACHE["nc"] = _build_bass()
    in_maps = _prep_in_maps(x, s, style_weight, style_bias, weight, bias)
    res = None
    for attempt in range(3):
        try:
            res = run_bass_kernel_spmd(_CACHE["nc"], in_maps, list(range(NCORES)))
            break
        except Exception:
            if attempt == 2:
                raise
            time.sleep(30)  # transient device wedge; recovers on its own
    LAST_RESULTS = res
    return _gather(res.results)



# revision 18
# speedup vs baseline: 1.2197x; 1.2197x over previous
"""ConvMod3d (StyleGAN-style modulated 3x3x3 conv, N=4 groups) on 8 trn2 cores.

Sharding: 8 shards = 4 samples x 2 depth-halves. Each core convolves a
25-plane input slab (64ch x 48x48) against its sample's modulated 64x64x27
weights, producing 23 output planes. Style modulation/demodulation of the
tiny weight tensor happens on host; the conv (99.8% of FLOPs) on device.

Per output plane d': 27 taps, each a [Cin=64 -> Cout=64] matmul over the
flattened 48x48 plane with a shifted read offset; invalid edge columns
(w'>=46, h'>=46) are computed and discarded on the host side.

PE packing (trn2 constraints: row tiling crashes the device; alternating
contract sizes back-to-back costs 2.2x, so same-contract matmuls are kept
contiguous). Taps are packed two-per-matmul on the contraction dim via
stacked SBUF windows:
- W[p]  = plane p (partitions 0-63) | plane p+1 (64-127): fuses the
  (kd=0,kd=1) tap pairs -> 9 contract-128 streams per output plane.
- W2[p] = plane p | plane p shifted +48 cols (one h row): fuses the
  (kd=2, kh=0/1) pairs -> 3 contract-128 streams; the 3 (kd=2,kh=2)
  taps stay contract-64 on W2's lower half. (A third +1-col-shift
  family packing those too was tried and lost: its extra HBM traffic
  stalls the PE on window waits.)
Two output planes run concurrently on PE col strips (plane A accumulates
in one PSUM bank partitions 0-63, plane B in another, partitions 64-127).
Matmuls in bf16 (fp32 PSUM accumulation).
"""

import time

import numpy as np
import ml_dtypes

import concourse.bacc as bacc
import concourse.bass as bass
import concourse.tile as tile
from concourse import mybir
from concourse.bass_utils import run_bass_kernel_spmd

EPS = 1e-8
N, CIN, COUT = 4, 64, 64
DHW, K = 48, 3
DOUT = DHW - K + 1          # 46
HALF = DOUT // 2            # 23 output planes per core
P_IN = HALF + K - 1         # 25 input planes per core
PLANE = DHW * DHW           # 2304
PAD_COLS = 192              # tail slack so shifted reads stay in-bounds
XS_COLS = P_IN * PLANE + PAD_COLS
WCOLS = PLANE + PAD_COLS - 64   # window columns (2432); max offset used 98+2207
PLANE_OUT = (DHW - 2) * DHW     # 2208 computed output cols (h' rows 0-45)
NTAPS = K * K * K           # 27
GROUP = 2                   # output planes per group (PSUM col strips)
NGROUPS = (HALF + GROUP - 1) // GROUP
CHUNKS = [(0, 512), (512, 512), (1024, 512), (1536, 512), (2048, 160)]
NCORES = 8
NWBLK = 15                  # weight blocks of 64 cols

F32 = mybir.dt.float32
MM_DT = mybir.dt.bfloat16
NP_MM = np.dtype(ml_dtypes.bfloat16)

_CACHE = {}
LAST_RESULTS = None  # BassKernelResults of the most recent device run


def _build_bass():
    nc = bacc.Bacc()
    xs = nc.declare_dram_parameter("xs", [CIN, XS_COLS], MM_DT, isOutput=False)
    wt = nc.declare_dram_parameter("wt", [128, NWBLK * COUT], MM_DT, isOutput=False)
    bt = nc.declare_dram_parameter("bt", [128, 1], F32, isOutput=False)
    y = nc.declare_dram_parameter(
        "y", [NGROUPS, GROUP * 64, PLANE_OUT], F32, isOutput=True)

    with tile.TileContext(nc) as tc:
        with (
            tc.tile_pool(name="const", bufs=1) as cpool,
            tc.tile_pool(name="xpool", bufs=16) as xpool,
            tc.tile_pool(name="opool", bufs=3) as opool,
            tc.tile_pool(name="ppool", bufs=8, space="PSUM") as ppool,
        ):
            wtile = cpool.tile([128, NWBLK * COUT], MM_DT)
            nc.sync.dma_start(out=wtile[:, :], in_=wt[:, :])
            btile = cpool.tile([128, 1], F32)
            nc.sync.dma_start(out=btile[:, :], in_=bt[:, :])

            windows = {}

            UPSHIFT = {"w": PLANE, "w2": DHW}

            def load_window(fam, p, split=False):
                # upper half holds the lower plane shifted by UPSHIFT[fam].
                # split=True loads in column halves so early matmuls (which
                # only touch low columns) start before the full window lands.
                key = (fam, p)
                if key in windows or p >= P_IN:
                    return
                xw = xpool.tile([128, WCOLS], MM_DT, tag="xw", name="xw")
                base = p * PLANE
                up = base + UPSHIFT[fam]
                cuts = [0, 1280, WCOLS] if split else [0, WCOLS]
                for a, b in zip(cuts, cuts[1:]):
                    nc.sync.dma_start(out=xw[0:64, a:b],
                                      in_=xs[:, base + a:base + b])
                    if up + WCOLS <= XS_COLS:
                        nc.sync.dma_start(out=xw[64:128, a:b],
                                          in_=xs[:, up + a:up + b])
                windows[key] = xw

            def ensure_group_windows(g, split=False):
                if g >= NGROUPS:
                    return
                for d in range(g * GROUP, min(HALF, (g + 1) * GROUP)):
                    load_window("w", d, split=split)
                    load_window("w2", d + 2, split=split)

            # Group-0 windows: issue ONLY the low-column cut before the
            # first matmuls. A matmul's DMA wait is a coarse semaphore
            # threshold covering every DMA issued before it in program
            # order, so anything issued earlier delays the first matmul
            # (measured: 2.9MB up-front -> first MM at ~13us). The rest of
            # g0's columns and the g1/g2/g3 prefetches are issued between
            # the first group's chunk iterations instead.
            XCUT = 1280
            g0wins = [("w", 0), ("w", 1), ("w2", 2), ("w2", 3)]
            for fam, p in g0wins:
                windows[(fam, p)] = xpool.tile([128, WCOLS], MM_DT,
                                               tag="xw", name="xw")

            def g0_cut(a, b):
                for fam, p in g0wins:
                    xw = windows[(fam, p)]
                    base = p * PLANE
                    up = base + UPSHIFT[fam]
                    nc.sync.dma_start(out=xw[0:64, a:b],
                                      in_=xs[:, base + a:base + b])
                    if up + WCOLS <= XS_COLS:
                        nc.sync.dma_start(out=xw[64:128, a:b],
                                          in_=xs[:, up + a:up + b])

            g0_cut(0, XCUT)

            for grp in range(NGROUPS):
                dps = [d for d in range(grp * GROUP, (grp + 1) * GROUP)
                       if d < HALF]
                if grp > 0:
                    ensure_group_windows(grp + 3)
                nparts = 64 * len(dps)

                ot = opool.tile([128, PLANE_OUT], F32, tag="ot")
                for cidx, (c0, csz) in enumerate(CHUNKS):
                    pss = [ppool.tile([128, 512], F32, tag="ps", name="ps")
                           for _ in dps]
                    # j 0-8 fused kd01 (c128, W[dp], off kh*48+kw);
                    # j 9-11 fused kd2 kh01 (c128, W2[dp+2], off kw);
                    # j 12-14 kd2 kh2 (c64, W2[dp+2] lower, off 96+kw).
                    # Same-contract matmuls contiguous; serpentine the
                    # kind order across chunks so chunk boundaries don't
                    # add a contract-size switch.
                    # (Explicit shared LDWEIGHTS per j was tried and lost
                    # badly: the Tile scheduler hoists dependency-free
                    # ldweights instructions to the front of the Tensor
                    # queue, and walrus ignores ldweights=False on
                    # InstMatmult, so every matmul self-loads anyway.)
                    jorder = list(range(NWBLK))
                    if cidx % 2 == 1:
                        jorder = jorder[12:] + jorder[:12]
                    for jj, j in enumerate(jorder):
                        rows = 128 if j < 12 else 64
                        for ci in range(len(dps)):
                            dst = pss[ci][ci * 64:(ci + 1) * 64, 0:csz]
                            if j < 9:
                                kh, kw = divmod(j, 3)
                                win = windows[("w", dps[ci])]
                                off = kh * DHW + kw + c0
                            elif j < 12:
                                kw = j - 9
                                win = windows[("w2", dps[ci] + 2)]
                                off = kw + c0
                            else:
                                kw = j - 12
                                win = windows[("w2", dps[ci] + 2)]
                                off = 2 * DHW + kw + c0
                            nc.tensor.matmul(
                                dst,
                                wtile[0:rows, j * 64:(j + 1) * 64],
                                win[0:rows, off:off + csz],
                                start=(jj == 0),
                                stop=(jj == NWBLK - 1),
                            )
                    if grp == 0:
                        # staged loads: anything issued before the first
                        # matmuls inflates their DMA-wait threshold, so
                        # the rest of the input streams in behind chunk 0
                        if cidx == 0:
                            g0_cut(XCUT, WCOLS)
                        elif cidx == 1:
                            ensure_group_windows(1)
                        elif cidx == 2:
                            ensure_group_windows(2)
                        elif cidx == 3:
                            ensure_group_windows(3)
                    for ci in range(len(dps)):
                        nc.scalar.activation(
                            ot[ci * 64:(ci + 1) * 64, c0:c0 + csz],
                            pss[ci][ci * 64:(ci + 1) * 64, 0:csz],
                            mybir.ActivationFunctionType.Identity,
                            bias=btile[ci * 64:(ci + 1) * 64, :],
                        )
                    if grp == NGROUPS - 1:
                        # last group: per-chunk store on the idle Vector
                        # queue so the final transfer overlaps the
                        # remaining chunks' matmuls
                        nc.gpsimd.dma_start(
                            out=y[grp, 0:nparts, c0:c0 + csz],
                            in_=ot[0:nparts, c0:c0 + csz])
                if grp < NGROUPS - 1:
                    nc.gpsimd.dma_start(out=y[grp, 0:nparts, :],
                                        in_=ot[0:nparts, :])
    nc.compile()
    return nc


def _prep_in_maps(x, s, style_weight, style_bias, weight, bias):
    style = s @ style_weight.T + style_bias                      # [N, Cin]
    wm = weight[None] * style[:, None, :, None, None, None]      # [N,Co,Ci,k,k,k]
    wm = wm * (1.0 / np.sqrt((wm * wm).sum(axis=(2, 3, 4, 5), keepdims=True) + EPS))
    wk = wm.transpose(0, 2, 3, 4, 5, 1)                          # [N,Ci,kd,kh,kw,Co]
    wfull = np.zeros((N, 128, NWBLK * COUT), np.float32)
    for j in range(9):
        kh, kw = divmod(j, 3)
        wfull[:, 0:64, j * 64:(j + 1) * 64] = wk[:, :, 0, kh, kw, :]
        wfull[:, 64:128, j * 64:(j + 1) * 64] = wk[:, :, 1, kh, kw, :]
    for kw in range(3):
        j = 9 + kw
        wfull[:, 0:64, j * 64:(j + 1) * 64] = wk[:, :, 2, 0, kw, :]
        wfull[:, 64:128, j * 64:(j + 1) * 64] = wk[:, :, 2, 1, kw, :]
    for kw in range(3):
        j = 12 + kw
        wfull[:, 0:64, j * 64:(j + 1) * 64] = wk[:, :, 2, 2, kw, :]
    wfull = np.ascontiguousarray(wfull.astype(NP_MM))
    bt = np.ascontiguousarray(
        np.tile(bias[:, None], (2, 1)), dtype=np.float32)        # [128,1]

    in_maps = []
    for core in range(NCORES):
        n, h = divmod(core, 2)
        d0 = h * HALF
        xsl = x[n, :, d0:d0 + P_IN].reshape(CIN, P_IN * PLANE)
        xsl = np.concatenate(
            [xsl, np.zeros((CIN, PAD_COLS), np.float32)], axis=1)
        in_maps.append({
            "xs": np.ascontiguousarray(xsl.astype(NP_MM)),
            "wt": wfull[n],
            "bt": bt,
        })
    return in_maps


def _gather(results):
    y = np.empty((N, COUT, DOUT, DOUT, DOUT), np.float32)
    for core in range(NCORES):
        n, h = divmod(core, 2)
        planes = results[core]["y"].reshape(
            NGROUPS * GROUP, COUT, DHW - 2, DHW)[:HALF]
        y[n, :, h * HALF:(h + 1) * HALF] = (
            planes[:, :, :, :DOUT].transpose(1, 0, 2, 3))
    return y


def kernel(x, s, style_weight, style_bias, weight, bias):
    global LAST_RESULTS
    x = np.asarray(x, np.float32)
    s = np.asarray(s, np.float32)
    style_weight = np.asarray(style_weight, np.float32)
    style_bias = np.asarray(style_bias, np.float32)
    weight = np.asarray(weight, np.float32)
    bias = np.asarray(bias, np.float32)

    if "nc" not in _CACHE:
        _CACHE["nc"] = _build_bass()
    in_maps = _prep_in_maps(x, s, style_weight, style_bias, weight, bias)
    res = None
    for attempt in range(3):
        try:
            res = run_bass_kernel_spmd(_CACHE["nc"], in_maps, list(range(NCORES)))
            break
        except Exception:
            if attempt == 2:
                raise
            time.sleep(30)  # transient device wedge; recovers on its own
    LAST_RESULTS = res
    return _gather(res.results)



# revision 19
# speedup vs baseline: 1.2493x; 1.0242x over previous
"""ConvMod3d (StyleGAN-style modulated 3x3x3 conv, N=4 groups) on 8 trn2 cores.

Sharding: 8 shards = 4 samples x 2 H-halves (was 2 D-halves). Each core
convolves 48 full-depth slabs of 25 h-rows (64ch x 25x48) against its
sample's modulated 64x64x27 weights, producing all 46 output D-planes x 23
h'-rows. 46 planes = 23 EVEN pairs, so the PE col-strip pairing never
degrades to a half-rate single-plane group (the D-split's odd 23rd plane
cost a full-rate group span). Style modulation/demodulation on host; the
conv (99.8% of FLOPs) on device.

Per output plane d': 27 taps, each a [Cin=64 -> Cout=64] matmul over the
flattened (h,w) slab (25x48 -> window of 1216 cols) with a shifted read
offset; invalid edge columns (w'>=46) are computed and discarded on host.

PE packing (trn2 constraints: row tiling crashes the device; alternating
contract sizes back-to-back costs 2.2x). Taps packed two-per-matmul on the
contraction dim via stacked SBUF windows:
- W[d]  = slab d (partitions 0-63) | slab d+1 (64-127): fuses the
  (kd=0,kd=1) tap pairs -> 9 contract-128 streams per output plane.
- W2[d] = slab d | slab d shifted +48 cols (one h row): fuses the
  (kd=2, kh=0/1) pairs -> 3 contract-128 streams; the 3 (kd=2,kh=2)
  taps stay contract-64 on W2's lower half.
Two output planes run concurrently on PE col strips. Matmuls in bf16
(fp32 PSUM accumulation).

Head: only the first pair's low window columns are DMA'd before the first
matmuls (a matmul's DMA wait is a coarse semaphore threshold covering
every DMA issued before it); the rest stream in behind pair-0's chunks.
"""

import time

import numpy as np
import ml_dtypes

import concourse.bacc as bacc
import concourse.bass as bass
import concourse.tile as tile
from concourse import mybir
from concourse.bass_utils import run_bass_kernel_spmd

EPS = 1e-8
N, CIN, COUT = 4, 64, 64
DHW, K = 48, 3
DOUT = DHW - K + 1          # 46 output planes per core (full depth)
H_HALF = DOUT // 2          # 23 output h'-rows per core
H_IN = H_HALF + K - 1       # 25 input h-rows per core
PLANE = H_IN * DHW          # 1200 cols per slab (25 h-rows x 48 w)
PAD_COLS = 192              # tail slack so shifted reads stay in-bounds
XS_COLS = DHW * PLANE + PAD_COLS   # 48 slabs
WCOLS = 1216                # window columns; max offset used 98+1103
PLANE_OUT = H_HALF * DHW    # 1104 computed output cols (23 h'-rows x 48)
NPAIRS = DOUT // 2          # 23 output-plane pairs
CHUNKS = [(0, 512), (512, 512), (1024, 80)]
NCORES = 8
NWBLK = 15                  # weight blocks of 64 cols
XCUT = 640                  # first-cut columns (covers chunk 0: off<=610)

F32 = mybir.dt.float32
MM_DT = mybir.dt.bfloat16
NP_MM = np.dtype(ml_dtypes.bfloat16)

_CACHE = {}
LAST_RESULTS = None  # BassKernelResults of the most recent device run


def _build_bass():
    nc = bacc.Bacc()
    xs = nc.declare_dram_parameter("xs", [CIN, XS_COLS], MM_DT, isOutput=False)
    wt = nc.declare_dram_parameter("wt", [128, NWBLK * COUT], MM_DT, isOutput=False)
    bt = nc.declare_dram_parameter("bt", [128, 1], F32, isOutput=False)
    y = nc.declare_dram_parameter(
        "y", [NPAIRS, 128, PLANE_OUT], F32, isOutput=True)

    with tile.TileContext(nc) as tc:
        with (
            tc.tile_pool(name="const", bufs=1) as cpool,
            tc.tile_pool(name="xpool", bufs=16) as xpool,
            tc.tile_pool(name="opool", bufs=3) as opool,
            tc.tile_pool(name="ppool", bufs=8, space="PSUM") as ppool,
        ):
            wtile = cpool.tile([128, NWBLK * COUT], MM_DT)
            nc.sync.dma_start(out=wtile[:, :], in_=wt[:, :])
            btile = cpool.tile([128, 1], F32)
            nc.sync.dma_start(out=btile[:, :], in_=bt[:, :])

            windows = {}

            UPSHIFT = {"w": PLANE, "w2": DHW}

            def load_window(fam, p, a=0, b=WCOLS, engs=None):
                key = (fam, p)
                if key in windows:
                    xw = windows[key]
                else:
                    xw = xpool.tile([128, WCOLS], MM_DT, tag="xw", name="xw")
                    windows[key] = xw
                base = p * PLANE
                up = base + UPSHIFT[fam]
                e1, e2 = engs if engs else (nc.sync, nc.sync)
                e1.dma_start(out=xw[0:64, a:b],
                             in_=xs[:, base + a:base + b])
                e2.dma_start(out=xw[64:128, a:b],
                             in_=xs[:, up + a:up + b])

            def ensure_pair(p):
                if p >= NPAIRS:
                    return
                for d in (2 * p, 2 * p + 1):
                    if ("w", d) not in windows:
                        load_window("w", d)
                    if ("w2", d + 2) not in windows:
                        load_window("w2", d + 2)

            # PE warm-up: a few throwaway matmuls on the (first-loaded)
            # weight tile trip the HAM clock gate to K=8/8 while the first
            # windows stream in; without them the first ~5us of real
            # matmuls run at the 1.2GHz cold clock. Scratch PSUM, never
            # read.
            wps = ppool.tile([128, 512], F32, tag="ps", name="ps")
            for _ in range(5):
                nc.tensor.matmul(
                    wps[0:64, 0:512],
                    wtile[0:128, 0:64],
                    wtile[0:128, 0:512],
                    start=True, stop=True,
                )

            # pair 0: low columns only before the first matmuls, issue
            # spread across the three DMA-capable queues — each
            # DMA_DIRECT2D issue costs ~600ns and the queues run them
            # serially, so single-queue issue alone costs ~5us of head
            p0wins = [("w", 0), ("w", 1), ("w2", 2), ("w2", 3)]
            issue_engs = [(nc.scalar, nc.gpsimd), (nc.sync, nc.scalar),
                          (nc.gpsimd, nc.sync), (nc.scalar, nc.gpsimd)]
            for (fam, p), engs in zip(p0wins, issue_engs):
                load_window(fam, p, 0, XCUT, engs=engs)

            for grp in range(NPAIRS):
                dps = [2 * grp, 2 * grp + 1]
                if grp > 0:
                    ensure_pair(grp + 3)

                ot = opool.tile([128, PLANE_OUT], F32, tag="ot")
                for cidx, (c0, csz) in enumerate(CHUNKS):
                    pss = [ppool.tile([128, 512], F32, tag="ps", name="ps")
                           for _ in dps]
                    # j 0-8 fused kd01 (c128, W[dp], off kh*48+kw);
                    # j 9-11 fused kd2 kh01 (c128, W2[dp+2], off kw);
                    # j 12-14 kd2 kh2 (c64, W2[dp+2] lower, off 96+kw).
                    # Same-contract matmuls contiguous; serpentine the
                    # kind order across chunks so chunk boundaries don't
                    # add a contract-size switch.
                    jorder = list(range(NWBLK))
                    if cidx % 2 == 1:
                        jorder = jorder[12:] + jorder[:12]
                    for jj, j in enumerate(jorder):
                        rows = 128 if j < 12 else 64
                        for ci in range(len(dps)):
                            dst = pss[ci][ci * 64:(ci + 1) * 64, 0:csz]
                            if j < 9:
                                kh, kw = divmod(j, 3)
                                win = windows[("w", dps[ci])]
                                off = kh * DHW + kw + c0
                            elif j < 12:
                                kw = j - 9
                                win = windows[("w2", dps[ci] + 2)]
                                off = kw + c0
                            else:
                                kw = j - 12
                                win = windows[("w2", dps[ci] + 2)]
                                off = 2 * DHW + kw + c0
                            nc.tensor.matmul(
                                dst,
                                wtile[0:rows, j * 64:(j + 1) * 64],
                                win[0:rows, off:off + csz],
                                start=(jj == 0),
                                stop=(jj == NWBLK - 1),
                            )
                    for ci in range(len(dps)):
                        nc.scalar.activation(
                            ot[ci * 64:(ci + 1) * 64, c0:c0 + csz],
                            pss[ci][ci * 64:(ci + 1) * 64, 0:csz],
                            mybir.ActivationFunctionType.Identity,
                            bias=btile[ci * 64:(ci + 1) * 64, :],
                        )
                    if grp == NPAIRS - 1:
                        # last pair: per-chunk store on the idle GpSimd
                        # queue so the final transfer overlaps the
                        # remaining chunks' matmuls
                        nc.gpsimd.dma_start(
                            out=y[grp, :, c0:c0 + csz],
                            in_=ot[:, c0:c0 + csz])
                    if grp == 0:
                        # staged loads: anything issued before the first
                        # matmuls inflates their DMA-wait threshold
                        if cidx == 0:
                            for fam, p in p0wins:
                                load_window(fam, p, XCUT, WCOLS)
                            ensure_pair(1)
                        elif cidx == 1:
                            ensure_pair(2)
                        elif cidx == 2:
                            ensure_pair(3)
                if grp < NPAIRS - 1:
                    nc.gpsimd.dma_start(out=y[grp, :, :], in_=ot[:, :])
    nc.compile()
    return nc


def _prep_in_maps(x, s, style_weight, style_bias, weight, bias):
    style = s @ style_weight.T + style_bias                      # [N, Cin]
    wm = weight[None] * style[:, None, :, None, None, None]      # [N,Co,Ci,k,k,k]
    wm = wm * (1.0 / np.sqrt((wm * wm).sum(axis=(2, 3, 4, 5), keepdims=True) + EPS))
    wk = wm.transpose(0, 2, 3, 4, 5, 1)                          # [N,Ci,kd,kh,kw,Co]
    wfull = np.zeros((N, 128, NWBLK * COUT), np.float32)
    for j in range(9):
        kh, kw = divmod(j, 3)
        wfull[:, 0:64, j * 64:(j + 1) * 64] = wk[:, :, 0, kh, kw, :]
        wfull[:, 64:128, j * 64:(j + 1) * 64] = wk[:, :, 1, kh, kw, :]
    for kw in range(3):
        j = 9 + kw
        wfull[:, 0:64, j * 64:(j + 1) * 64] = wk[:, :, 2, 0, kw, :]
        wfull[:, 64:128, j * 64:(j + 1) * 64] = wk[:, :, 2, 1, kw, :]
    for kw in range(3):
        j = 12 + kw
        wfull[:, 0:64, j * 64:(j + 1) * 64] = wk[:, :, 2, 2, kw, :]
    wfull = np.ascontiguousarray(wfull.astype(NP_MM))
    bt = np.ascontiguousarray(
        np.tile(bias[:, None], (2, 1)), dtype=np.float32)        # [128,1]

    in_maps = []
    for core in range(NCORES):
        n, h = divmod(core, 2)
        h0 = h * H_HALF
        xsl = x[n, :, :, h0:h0 + H_IN, :].reshape(CIN, DHW * PLANE)
        xsl = np.concatenate(
            [xsl, np.zeros((CIN, PAD_COLS), np.float32)], axis=1)
        in_maps.append({
            "xs": np.ascontiguousarray(xsl.astype(NP_MM)),
            "wt": wfull[n],
            "bt": bt,
        })
    return in_maps


def _gather(results):
    y = np.empty((N, COUT, DOUT, DOUT, DOUT), np.float32)
    for core in range(NCORES):
        n, h = divmod(core, 2)
        planes = results[core]["y"].reshape(
            NPAIRS, 2, COUT, H_HALF, DHW)       # [pair, ci, co, h', w]
        full = planes.transpose(2, 0, 1, 3, 4).reshape(
            COUT, DOUT, H_HALF, DHW)            # [co, d'=2p+ci, h', w]
        y[n, :, :, h * H_HALF:(h + 1) * H_HALF, :] = full[:, :, :, :DOUT]
    return y


def kernel(x, s, style_weight, style_bias, weight, bias):
    global LAST_RESULTS
    x = np.asarray(x, np.float32)
    s = np.asarray(s, np.float32)
    style_weight = np.asarray(style_weight, np.float32)
    style_bias = np.asarray(style_bias, np.float32)
    weight = np.asarray(weight, np.float32)
    bias = np.asarray(bias, np.float32)

    if "nc" not in _CACHE:
        _CACHE["nc"] = _build_bass()
    in_maps = _prep_in_maps(x, s, style_weight, style_bias, weight, bias)
    res = None
    for attempt in range(3):
        try:
            res = run_bass_kernel_spmd(_CACHE["nc"], in_maps, list(range(NCORES)))
            break
        except Exception:
            if attempt == 2:
                raise
            time.sleep(30)  # transient device wedge; recovers on its own
    LAST_RESULTS = res
    return _gather(res.results)
